# revision 24
# baseline (speedup 1.0000x reference)
"""MoE routed conv for Trainium2, 8-core SPMD.

Math: each batch image selects one expert (argmax of scores); the forward
output equals a 3x3 pad-1 conv of that image with the selected expert's
[128,128,3,3] filter (the dense conv + one-hot mask in the reference).
So we compute only the selected expert's conv: 5x less work.

Distribution: data-parallel over batch, 4 images per core. Host does the
(trivial) argmax routing + weight gather; the device program is uniform SPMD.

Device kernel (per core): shifted-window implicit GEMM in bf16.
  - bf16 operands stream 1 col/cycle at 2.4GHz (same rate as fp32r) but
    halve every DMA (x, w, out) and enable fast-weight-load so LDWEIGHTS
    hides under the previous matmul.
  - Warmup matmuls on zeroed scratch run during the ~2us DMA head so the
    PE's HAM clock gate reaches 8/8 before real data lands.
  - For each of 9 taps (kh,kw), one matmul per 8-row output chunk
    accumulates into PSUM: psum[co, h, w] += w_tap[ci,co].T @ xpad[...].
  - Output staged to SBUF as bf16 (DVE copy casts), host upcasts to f32.
"""
import numpy as np

B, C, H, W = 32, 128, 56, 56
E, OC = 5, 128
NCORES = 8
IPC = B // NCORES          # images per core
CH = 8                     # output rows per chunk
NCHUNK = H // CH           # 7
WP = W + 2                 # padded width
NWARM = 26                 # PE warmup matmuls (run during DMA head)
WARMN = 112                # warmup matmul free size (cols)

_program = None


def _build_program():
    import concourse.bacc as bacc
    import concourse.tile as tile
    from concourse.tile import add_dep_helper
    from concourse import mybir

    dt = mybir.dt
    idt = dt.bfloat16
    nc = bacc.Bacc("TRN2", target_bir_lowering=False, debug=False)
    x_d = nc.dram_tensor("x", [IPC, C, H, WP], idt, kind="ExternalInput").ap()
    w_d = nc.dram_tensor("w", [IPC, C, 9, OC], idt, kind="ExternalInput").ap()
    o_d = nc.dram_tensor("o", [IPC, OC, H, W], idt, kind="ExternalOutput").ap()

    NXT = 3  # x-tile ring depth

    with tile.TileContext(nc) as tc:
        with (
            tc.tile_pool(name="xp", bufs=1) as xp,
            tc.tile_pool(name="wpool", bufs=1) as wpool,
            tc.tile_pool(name="opool", bufs=1) as opool,
            tc.tile_pool(name="ps", bufs=8, space="PSUM") as psp,
        ):
            xts = [xp.tile([C, H, WP], idt, name=f"xt{i}") for i in range(NXT)]
            wts = [wpool.tile([C, 9, OC], idt, name=f"wt{i}") for i in range(IPC)]
            ots = [opool.tile([OC, H, W], idt, name=f"ot{i}") for i in range(2)]

            # PE warmup: matmuls on zeroed scratch, no DMA deps, so they
            # run during the input-DMA head and lift HAM to 8/8 before
            # real data lands.
            wsc = wpool.tile([C, OC], idt, name="wsc")
            xsc = xp.tile([C, WARMN], idt, name="xsc")
            nc.vector.memset(wsc[:], 0)
            nc.gpsimd.memset(xsc[:], 0)
            psw = psp.tile([OC, WARMN], dt.float32, name="psw", tag="ps")
            # small free dim, back-to-back: bridges PE-ready (~7us) to
            # data-ready (~10us) with continuous activity so the HAM clock
            # gate reaches 8/8 with no idle window in between
            for i in range(NWARM):
                nc.tensor.matmul(psw[:], wsc[:], xsc[:],
                                 start=(i == 0), stop=(i == NWARM - 1))

            anchor = None  # gates img>=1 prefetch DMAs off the head's critical path
            for img in range(IPC):
                xt = xts[img % NXT]
                wt = wts[img]
                ot = ots[img % 2]
                if img == 0:
                    # split w so the first taps' weights land before the
                    # whole 295KB transfer completes
                    loads = [nc.sync.dma_start(out=wt[:, 0:4, :], in_=w_d[img, :, 0:4, :]),
                             nc.sync.dma_start(out=wt[:, 4:9, :], in_=w_d[img, :, 4:9, :])]
                else:
                    loads = [nc.sync.dma_start(out=wt[:], in_=w_d[img])]
                if img == 0:
                    # image 0: pieces serialized on one ring so they complete
                    # in consumption order (parallel rings round-robin bytes
                    # and delay the first piece); first piece small to start
                    # compute early
                    xsegs = [(0, 9), (9, 17), (17, 33), (33, 56)]
                else:
                    xsegs = [(0, 56)]
                for (ra, rb) in xsegs:
                    loads.append(nc.scalar.dma_start(
                        out=xt[:, ra:rb, :], in_=x_d[img, :, ra:rb, :]))
                if img >= 1 and anchor is not None:
                    for ld in loads:
                        add_dep_helper(ld.ins, anchor.ins, sync=True,
                                       reason="delay prefetch past head-critical DMAs")

                last_img = img == IPC - 1
                subchunks = [(c * CH, CH) for c in range(NCHUNK)]
                if last_img:
                    # split the final chunk so earlier pieces' flushes overlap
                    # later pieces' matmuls; last piece tiny -> short serial
                    # tail (CAST + DMA of 2 rows)
                    subchunks = subchunks[:-1] + [(48, 4), (52, 2), (54, 1), (55, 1)]
                for c, (r0, ch) in enumerate(subchunks):
                    ps = psp.tile([OC, ch, W], dt.float32, name=f"ps{img}_{c}", tag="ps")
                    taps = []
                    for kh in range(3):
                        for kw in range(3):
                            hs = max(r0, 1 - kh)
                            he = min(r0 + ch, H + 1 - kh)
                            if he > hs:  # fully clipped taps hit only padding
                                taps.append((kh, kw, hs, he))
                    for i, (kh, kw, hs, he) in enumerate(taps):
                        rhs = xt[:, hs + kh - 1 : he + kh - 1, kw : kw + W]
                        out = ps[:, hs - r0 : he - r0, :]
                        mm = nc.tensor.matmul(out, wt[:, kh * 3 + kw, :], rhs,
                                              start=(i == 0), stop=(i == len(taps) - 1))
                    if c == 0:
                        anchor = mm
                    if not last_img:
                        nc.vector.tensor_copy(ot[:, r0 : r0 + ch, :], ps[:])
                        if r0 + ch == 32:
                            nc.sync.dma_start(out=o_d[img, :, 0:32, :], in_=ot[:, 0:32, :])
                        elif r0 + ch == 56:
                            nc.sync.dma_start(out=o_d[img, :, 32:56, :], in_=ot[:, 32:56, :])
                    else:
                        # last image: fewer, staged flushes to shorten the
                        # tail without paying a 600ns DMA trigger per chunk
                        nc.vector.tensor_copy(ot[:, r0 : r0 + ch, :], ps[:])
                        if r0 + ch == 32:
                            nc.sync.dma_start(out=o_d[img, :, 0:32, :],
                                              in_=ot[:, 0:32, :])
                        elif r0 + ch == 48:
                            nc.sync.dma_start(out=o_d[img, :, 32:48, :],
                                              in_=ot[:, 32:48, :])
                        elif r0 + ch == 52:
                            nc.sync.dma_start(out=o_d[img, :, 48:52, :],
                                              in_=ot[:, 48:52, :])
                        elif r0 + ch == 54:
                            # tail flushes on separate rings: triggers run in
                            # parallel, the final sync flush stays tiny
                            nc.scalar.dma_start(out=o_d[img, :, 52:54, :],
                                                in_=ot[:, 52:54, :])
                        elif r0 + ch == 55:
                            nc.gpsimd.dma_start(out=o_d[img, :, 54:55, :],
                                                in_=ot[:, 54:55, :])
                        elif r0 + ch == 56:
                            nc.sync.dma_start(out=o_d[img, :, 55:56, :],
                                              in_=ot[:, 55:56, :])
    nc.compile()
    return nc


def _get_program():
    global _program
    if _program is None:
        _program = _build_program()
    return _program


def kernel(x: np.ndarray, scores: np.ndarray, weight: np.ndarray,
           **run_kwargs) -> np.ndarray:
    import ml_dtypes
    from concourse.bass_utils import run_bass_kernel_spmd

    bf16 = ml_dtypes.bfloat16
    x = np.asarray(x, dtype=np.float32)
    scores = np.asarray(scores, dtype=np.float32)
    weight = np.asarray(weight, dtype=np.float32)

    expert = np.argmax(scores, axis=1)                       # [B]
    w_sel = weight.reshape(E, OC, C, 3, 3)[expert]           # [B, co, ci, kh, kw]
    # lhsT layout: [ci, tap, co]
    w_lhsT = np.ascontiguousarray(
        w_sel.transpose(0, 2, 3, 4, 1).reshape(B, C, 9, OC)).astype(bf16)
    xpad = np.zeros((B, C, H, WP), bf16)
    xpad[:, :, :, 1 : W + 1] = x.astype(bf16)

    nc = _get_program()
    in_maps = [
        {"x": xpad[k * IPC : (k + 1) * IPC], "w": w_lhsT[k * IPC : (k + 1) * IPC]}
        for k in range(NCORES)
    ]
    res = run_bass_kernel_spmd(nc, in_maps, list(range(NCORES)), **run_kwargs)
    out = np.concatenate([res.results[k]["o"] for k in range(NCORES)], axis=0)
    if run_kwargs:
        kernel.last_results = res
    return out.astype(np.float32)


# revision 26
# speedup vs baseline: 1.2017x; 1.2017x over previous
"""MoE routed conv for Trainium2, 8-core SPMD.

Math: each batch image selects one expert (argmax of scores); the forward
output equals a 3x3 pad-1 conv of that image with the selected expert's
[128,128,3,3] filter (the dense conv + one-hot mask in the reference).
So we compute only the selected expert's conv: 5x less work.

Distribution: data-parallel over batch, 4 images per core. Host does the
(trivial) argmax routing + weight gather; the device program is uniform SPMD.

Device kernel (per core): shifted-window implicit GEMM in bf16.
  - bf16 operands stream 1 col/cycle at 2.4GHz (same rate as fp32r) but
    halve every DMA (x, w, out) and enable fast-weight-load so LDWEIGHTS
    hides under the previous matmul.
  - Warmup matmuls on zeroed scratch run during the ~2us DMA head so the
    PE's HAM clock gate reaches 8/8 before real data lands.
  - For each of 9 taps (kh,kw), one matmul per 8-row output chunk
    accumulates into PSUM: psum[co, h, w] += w_tap[ci,co].T @ xpad[...].
  - Output staged to SBUF as bf16 (DVE copy casts), host upcasts to f32.
"""
import numpy as np

B, C, H, W = 32, 128, 56, 56
E, OC = 5, 128
NCORES = 8
IPC = B // NCORES          # images per core
CH = 8                     # output rows per chunk
NCHUNK = H // CH           # 7
WP = W + 2                 # padded width
NWARM = 26                 # PE warmup matmuls (run during DMA head)
WARMN = 112                # warmup matmul free size (cols)

_program = None


def _build_program():
    import concourse.bacc as bacc
    import concourse.tile as tile
    from concourse.tile import add_dep_helper
    from concourse import mybir

    dt = mybir.dt
    idt = dt.bfloat16
    nc = bacc.Bacc("TRN2", target_bir_lowering=False, debug=False)
    x_d = nc.dram_tensor("x", [IPC, C, H, WP], idt, kind="ExternalInput").ap()
    w_d = nc.dram_tensor("w", [IPC, C, 9, OC], idt, kind="ExternalInput").ap()
    o_d = nc.dram_tensor("o", [IPC, OC, H, W], idt, kind="ExternalOutput").ap()

    NXT = 3  # x-tile ring depth

    with tile.TileContext(nc) as tc:
        with (
            tc.tile_pool(name="xp", bufs=1) as xp,
            tc.tile_pool(name="wpool", bufs=1) as wpool,
            tc.tile_pool(name="opool", bufs=1) as opool,
            tc.tile_pool(name="ps", bufs=8, space="PSUM") as psp,
        ):
            xts = [xp.tile([C, H, WP], idt, name=f"xt{i}") for i in range(NXT)]
            wts = [wpool.tile([C, 9, OC], idt, name=f"wt{i}") for i in range(IPC)]
            ots = [opool.tile([OC, H, W], idt, name=f"ot{i}") for i in range(2)]

            # PE warmup: matmuls on zeroed scratch, no DMA deps, so they
            # run during the input-DMA head and lift HAM to 8/8 before
            # real data lands.
            wsc = wpool.tile([C, OC], idt, name="wsc")
            xsc = xp.tile([C, WARMN], idt, name="xsc")
            nc.vector.memset(wsc[:], 0)
            nc.gpsimd.memset(xsc[:], 0)
            psw = psp.tile([OC, WARMN], dt.float32, name="psw", tag="ps")
            # small free dim, back-to-back: bridges PE-ready (~7us) to
            # data-ready (~10us) with continuous activity so the HAM clock
            # gate reaches 8/8 with no idle window in between
            for i in range(NWARM):
                nc.tensor.matmul(psw[:], wsc[:], xsc[:],
                                 start=(i == 0), stop=(i == NWARM - 1))

            anchor = None  # gates img>=1 prefetch DMAs off the head's critical path
            for img in range(IPC):
                xt = xts[img % NXT]
                wt = wts[img]
                ot = ots[img % 2]
                if img == 0:
                    # split w so the first taps' weights land before the
                    # whole 295KB transfer completes
                    loads = [nc.sync.dma_start(out=wt[:, 0:4, :], in_=w_d[img, :, 0:4, :]),
                             nc.sync.dma_start(out=wt[:, 4:9, :], in_=w_d[img, :, 4:9, :])]
                else:
                    loads = [nc.sync.dma_start(out=wt[:], in_=w_d[img])]
                if img == 0:
                    # image 0: pieces serialized on one ring so they complete
                    # in consumption order (parallel rings round-robin bytes
                    # and delay the first piece); first piece small to start
                    # compute early
                    xsegs = [(0, 9), (9, 17), (17, 33), (33, 56)]
                else:
                    xsegs = [(0, 56)]
                for (ra, rb) in xsegs:
                    loads.append(nc.scalar.dma_start(
                        out=xt[:, ra:rb, :], in_=x_d[img, :, ra:rb, :]))
                if img >= 1 and anchor is not None:
                    for ld in loads:
                        add_dep_helper(ld.ins, anchor.ins, sync=True,
                                       reason="delay prefetch past head-critical DMAs")

                last_img = img == IPC - 1
                subchunks = [(c * CH, CH) for c in range(NCHUNK)]
                if last_img:
                    # split the final chunk so earlier pieces' flushes overlap
                    # later pieces' matmuls; last piece tiny -> short serial
                    # tail (CAST + DMA of 2 rows)
                    subchunks = subchunks[:-1] + [(48, 4), (52, 2), (54, 2)]
                for c, (r0, ch) in enumerate(subchunks):
                    ps = psp.tile([OC, ch, W], dt.float32, name=f"ps{img}_{c}", tag="ps")
                    taps = []
                    for kh in range(3):
                        for kw in range(3):
                            hs = max(r0, 1 - kh)
                            he = min(r0 + ch, H + 1 - kh)
                            if he > hs:  # fully clipped taps hit only padding
                                taps.append((kh, kw, hs, he))
                    for i, (kh, kw, hs, he) in enumerate(taps):
                        rhs = xt[:, hs + kh - 1 : he + kh - 1, kw : kw + W]
                        out = ps[:, hs - r0 : he - r0, :]
                        mm = nc.tensor.matmul(out, wt[:, kh * 3 + kw, :], rhs,
                                              start=(i == 0), stop=(i == len(taps) - 1))
                    if c == 0:
                        anchor = mm
                    if not last_img:
                        nc.vector.tensor_copy(ot[:, r0 : r0 + ch, :], ps[:])
                        if r0 + ch == 32:
                            nc.sync.dma_start(out=o_d[img, :, 0:32, :], in_=ot[:, 0:32, :])
                        elif r0 + ch == 56:
                            nc.sync.dma_start(out=o_d[img, :, 32:56, :], in_=ot[:, 32:56, :])
                    else:
                        # last image: fewer, staged flushes to shorten the
                        # tail without paying a 600ns DMA trigger per chunk
                        nc.vector.tensor_copy(ot[:, r0 : r0 + ch, :], ps[:])
                        if r0 + ch == 32:
                            nc.sync.dma_start(out=o_d[img, :, 0:32, :],
                                              in_=ot[:, 0:32, :])
                        elif r0 + ch == 48:
                            nc.sync.dma_start(out=o_d[img, :, 32:48, :],
                                              in_=ot[:, 32:48, :])
                        elif r0 + ch == 52:
                            nc.sync.dma_start(out=o_d[img, :, 48:52, :],
                                              in_=ot[:, 48:52, :])
                        elif r0 + ch == 54:
                            # scalar ring: triggers in parallel with sync's
                            # [48:52], keeps the final sync flush tiny
                            nc.scalar.dma_start(out=o_d[img, :, 52:54, :],
                                                in_=ot[:, 52:54, :])
                        elif r0 + ch == 56:
                            nc.sync.dma_start(out=o_d[img, :, 54:56, :],
                                              in_=ot[:, 54:56, :])
    nc.compile()
    return nc


def _get_program():
    global _program
    if _program is None:
        _program = _build_program()
    return _program


def kernel(x: np.ndarray, scores: np.ndarray, weight: np.ndarray,
           **run_kwargs) -> np.ndarray:
    import ml_dtypes
    from concourse.bass_utils import run_bass_kernel_spmd

    bf16 = ml_dtypes.bfloat16
    x = np.asarray(x, dtype=np.float32)
    scores = np.asarray(scores, dtype=np.float32)
    weight = np.asarray(weight, dtype=np.float32)

    expert = np.argmax(scores, axis=1)                       # [B]
    w_sel = weight.reshape(E, OC, C, 3, 3)[expert]           # [B, co, ci, kh, kw]
    # lhsT layout: [ci, tap, co]
    w_lhsT = np.ascontiguousarray(
        w_sel.transpose(0, 2, 3, 4, 1).reshape(B, C, 9, OC)).astype(bf16)
    xpad = np.zeros((B, C, H, WP), bf16)
    xpad[:, :, :, 1 : W + 1] = x.astype(bf16)

    nc = _get_program()
    in_maps = [
        {"x": xpad[k * IPC : (k + 1) * IPC], "w": w_lhsT[k * IPC : (k + 1) * IPC]}
        for k in range(NCORES)
    ]
    res = run_bass_kernel_spmd(nc, in_maps, list(range(NCORES)), **run_kwargs)
    out = np.concatenate([res.results[k]["o"] for k in range(NCORES)], axis=0)
    if run_kwargs:
        kernel.last_results = res
    return out.astype(np.float32)
